# revision 53
# baseline (speedup 1.0000x reference)
"""Trainium2 Bass kernel for nn_MeshUpConv (3x chained SplineConv, deg-2 2D
B-spline, N=100k nodes, E=1.6M edges) on 8 NeuronCores.

Strategy (destination-bucketed graph parallel):
  - Host: bucket edges by destination-owner core; within a core sort by
    (source-block, local dst). Per (core, source-block) the edge count is
    padded to whole 128-edge tiles with counts equalized across cores so the
    SPMD program is shared.
  - Device, per layer:
      * node phase: every core computes the FULL node-level transform table
        xW [Npad, 640] (bf16 rows: 576 = [o-major, s-minor] + 64 zero pad)
        via PE matmuls into its own DRAM.
      * edge phase, per source-block g: dma_gather (int16 block-local src
        ids, 2048 tokens/instr) pulls xW rows; DVE multiplies by the 9
        spline basis products (broadcast AP) and group-reduces; per-tile
        selection-matrix matmuls (1 or 2 windows, straddle-aware)
        scatter-add 128-edge tiles into per-window PSUM, flushed into an
        SBUF accumulator slab shared across blocks. Root terms ride block
        0's PSUM. Final: ReLU (+transpose for the next layer's
        feature-major consumption).
      * AllGather (partition-concat) of transposed shard outputs between
        layers.
All sharding metadata (bucketing, sorting, padding, index staging) is host
side; all numerics (basis evaluation, matmuls, gathers, reductions) on
device.
"""
import sys

sys.path.insert(0, "/opt/trn_rl_repo")

from contextlib import ExitStack
from dataclasses import dataclass

import numpy as np

import concourse.bass as bass
import concourse.tile as tile
from concourse import mybir
from concourse.masks import make_identity

F32 = mybir.dt.float32
BF16 = mybir.dt.bfloat16
F16 = mybir.dt.float16
I16 = mybir.dt.int16
AF = mybir.ActivationFunctionType
OP = mybir.AluOpType

S = 9
CO = 64
ROWW = S * CO  # 576 payload row width
ROWP = 640  # padded table row (1280 B, %256)
WIN = 128


@dataclass
class Cfg:
    ncores: int = 8
    nsh: int = 12500  # real nodes per core
    nw: int = 98  # 128-dst windows per core
    cin: int = 128
    ch: int = 896  # node-chunk (divides shp, multiple of 128)
    ngrp: int = 4  # source block-pair groups
    gb: int = 12  # tiles per dma_gather (1536 tokens)
    bd: int = 4  # DVE batch (tiles, divides gb)
    tc_betas: int = 128  # betas chunk (tiles)

    @property
    def shp(self):
        return self.nw * WIN

    @property
    def npad(self):
        return self.ncores * self.shp

    @property
    def n(self):
        return self.ncores * self.nsh


FULL = Cfg()


# --------------------------------------------------------------------------
# host-side schedule / sharding
# --------------------------------------------------------------------------
def host_prep(cfg, x, skip, edge_index, edge_attr, W1, root1, W2, root2):
    ncores, nsh, nw, shp = cfg.ncores, cfg.nsh, cfg.nw, cfg.shp
    ngrp = cfg.ngrp
    bpg = ncores // ngrp  # blocks per group (2)
    grows = bpg * shp  # table rows per group (25088)
    nv = (nw + 3) // 4  # 512-dst windows (25)
    src = np.asarray(edge_index[0]).astype(np.int64)
    dst = np.asarray(edge_index[1]).astype(np.int64)
    attr = np.asarray(edge_attr, dtype=np.float32)
    owner = dst // nsh
    dloc = dst - owner * nsh
    sblk = src // nsh
    grp = sblk // bpg
    # group-local padded source row
    sloc = (sblk % bpg) * shp + (src - sblk * nsh)
    vwin = dloc // 512

    # counts per (core, group, v) -> equalized tile counts
    cnt = np.zeros((ncores, ngrp, nv), np.int64)
    for m in range(ncores):
        for g in range(ngrp):
            sel = (owner == m) & (grp == g)
            cnt[m, g] = np.bincount(vwin[sel], minlength=nv)
    tcv = np.maximum(1, -(-cnt.max(axis=0) // WIN)).astype(np.int64)  # [ngrp, nv]
    gtc = tcv.sum(axis=1)  # tiles per group
    goff = np.concatenate([[0], np.cumsum(gtc)]).astype(np.int64)
    ttot = int(goff[-1])
    ne = ttot * WIN
    # tile offset of (g, v)
    voff = np.zeros((ngrp, nv + 1), np.int64)
    for g in range(ngrp):
        voff[g] = np.concatenate([[goff[g]], goff[g] + np.cumsum(tcv[g])])

    zero_local = nsh  # block-local pad row (zero); lives in first block of grp
    srcs = np.full((ncores, ne), zero_local, np.int64)
    dst_rel = np.full((ncores, ne), -1, np.int64)
    attrs = np.zeros((ncores, ne, 2), np.float32)
    for m in range(ncores):
        own = owner == m
        for g in range(ngrp):
            ing = own & (grp == g)
            for v in range(nv):
                sel = np.where(ing & (vwin == v))[0]
                k = len(sel)
                base = int(voff[g, v]) * WIN
                srcs[m, base : base + k] = sloc[sel]
                dst_rel[m, base : base + k] = dloc[sel] - v * 512
                attrs[m, base : base + k] = attr[sel]
    # per-edge window-relative dst, fp16 (exact for 0..511), replicated 2x so
    # the on-device is_equal keeps a packed last dim (DVE 2x mode); pad
    # edges get -1 (matches nothing)
    dst2 = np.repeat(dst_rel.astype(np.float16)[:, :, None], 2, axis=2)

    def tilize(a):
        a = a.reshape(ttot, WIN, *a.shape[1:])
        return np.ascontiguousarray(np.swapaxes(a, 0, 1))

    def idx_wrap(a):
        w = a.reshape(-1, 16).T.astype(np.int16)
        return np.ascontiguousarray(np.tile(w, (8, 1)))

    srcs_w = np.stack([idx_wrap(srcs[m]) for m in range(ncores)])
    dstr_t = np.stack(
        [tilize(dst2[m]).reshape(WIN, ttot * 2) for m in range(ncores)]
    )
    attr_t = np.stack(
        [tilize(attrs[m]).reshape(WIN, ttot * 2) for m in range(ncores)]
    )

    cin = cfg.cin
    xpad = np.zeros((cfg.npad, cin), np.float32)
    spad = np.zeros((cfg.npad, CO), np.float32)
    for m in range(ncores):
        xpad[m * shp : m * shp + nsh] = x[m * nsh : (m + 1) * nsh]
        spad[m * shp : m * shp + nsh] = skip[m * nsh : (m + 1) * nsh]
    xT = np.ascontiguousarray(xpad.T)
    skipT = np.ascontiguousarray(spad.T)

    def wall(W, fdim):
        w = np.transpose(np.asarray(W, np.float32), (1, 2, 0)).reshape(fdim, ROWW)
        return np.ascontiguousarray(
            np.concatenate([w, np.zeros((fdim, ROWP - ROWW), np.float32)], axis=1)
        )

    W1all = wall(W1, cin)
    W2all = wall(W2, CO)
    iota512 = np.ascontiguousarray(
        np.tile(np.arange(512, dtype=np.float16), (WIN, 1))
    )

    shared = dict(
        xT=xT,
        skipT=skipT,
        W1all=W1all,
        W2all=W2all,
        root1=np.asarray(root1, np.float32),
        root2=np.asarray(root2, np.float32),
        iota=iota512,
    )
    in_maps = []
    for m in range(ncores):
        d = dict(shared)
        d["xTown"] = np.ascontiguousarray(xT[:, m * shp : (m + 1) * shp])
        d["skipTown"] = np.ascontiguousarray(skipT[:, m * shp : (m + 1) * shp])
        d["srcs"] = srcs_w[m]
        d["dstr"] = dstr_t[m]
        d["attr2"] = attr_t[m]
        in_maps.append(d)
    sched = dict(
        ttot=ttot,
        tcv=[[int(v) for v in row] for row in tcv],
        voff=[[int(v) for v in row] for row in voff],
    )
    return in_maps, sched


# --------------------------------------------------------------------------
# device program
# --------------------------------------------------------------------------
def build_program(cfg, sched):
    from concourse import bacc

    nc = bacc.Bacc(
        "TRN2",
        target_bir_lowering=False,
        debug=False,
        num_devices=cfg.ncores,
        num_swdge_queues=4,
    )
    cin, shp, npad, nw = cfg.cin, cfg.shp, cfg.npad, cfg.nw
    ch = cfg.ch
    ntch = ch // WIN
    assert shp % ch == 0
    nchunks = shp // ch
    ttot = sched["ttot"]
    tcv = sched["tcv"]
    voff = sched["voff"]
    ngrp = cfg.ngrp
    grows = (cfg.ncores // ngrp) * shp
    nv = (nw + 3) // 4

    xT = nc.declare_dram_parameter("xT", [cin, npad], F32, isOutput=False)
    xTown = nc.declare_dram_parameter("xTown", [cin, shp], F32, isOutput=False)
    skipT = nc.declare_dram_parameter("skipT", [CO, npad], F32, isOutput=False)
    skipTown = nc.declare_dram_parameter("skipTown", [CO, shp], F32, isOutput=False)
    W1all_d = nc.declare_dram_parameter("W1all", [cin, ROWP], F32, isOutput=False)
    W2all_d = nc.declare_dram_parameter("W2all", [CO, ROWP], F32, isOutput=False)
    root1_d = nc.declare_dram_parameter("root1", [cin, CO], F32, isOutput=False)
    root2_d = nc.declare_dram_parameter("root2", [CO, CO], F32, isOutput=False)
    iota_d = nc.declare_dram_parameter("iota", [WIN, 512], F16, isOutput=False)
    srcs_d = nc.declare_dram_parameter(
        "srcs", [WIN, (ttot * WIN) // 16], I16, isOutput=False
    )
    dstr_d = nc.declare_dram_parameter("dstr", [WIN, ttot * 2], F16, isOutput=False)
    attr2_d = nc.declare_dram_parameter(
        "attr2", [WIN, ttot * 2], F32, isOutput=False
    )
    out_d = nc.declare_dram_parameter("out_shard", [shp, CO], F32, isOutput=True)

    tables = [nc.dram_tensor(f"table{i}", [npad, ROWP], BF16) for i in range(3)]
    bounceT = [nc.dram_tensor(f"bounceT{i}", [CO, shp], BF16) for i in range(2)]
    hstackT = [
        nc.dram_tensor(
            f"hstackT{i}", [cfg.ncores * CO, shp], BF16, addr_space="Shared"
        )
        for i in range(2)
    ]
    skipTbf = nc.dram_tensor("skipTbf", [CO, npad], BF16)

    with tile.TileContext(nc) as tc:
        with ExitStack() as ctx:
            persist = ctx.enter_context(tc.tile_pool(name="persist", bufs=1))
            work = ctx.enter_context(tc.tile_pool(name="work", bufs=2))
            gpool = ctx.enter_context(tc.tile_pool(name="gpool", bufs=2))
            dpool = ctx.enter_context(tc.tile_pool(name="dpool", bufs=2))
            psum = ctx.enter_context(tc.tile_pool(name="psum", bufs=2, space="PSUM"))
            psum1 = ctx.enter_context(tc.tile_pool(name="psum1", bufs=1, space="PSUM"))
            psumw = ctx.enter_context(tc.tile_pool(name="psumw", bufs=2, space="PSUM"))

            # ------------- prep -------------
            iota_t = persist.tile([WIN, 512], F16, tag="iota")
            nc.sync.dma_start(iota_t[:], iota_d[:, :])
            ident = persist.tile([WIN, WIN], BF16, tag="ident")
            make_identity(nc, ident[:])

            def load_cvt(dram_ap, shape, tag):
                tmp = work.tile(shape, F32, tag="wcvt")
                nc.sync.dma_start(tmp[:], dram_ap)
                out = persist.tile(shape, BF16, tag=tag)
                nc.vector.tensor_copy(out[:], tmp[:])
                return out

            W1b = load_cvt(W1all_d[:, :], [cin, ROWP], "W1b")
            W2b = load_cvt(W2all_d[:, :], [CO, ROWP], "W2b")
            root1b = load_cvt(root1_d[:, :], [cin, CO], "root1b")
            root1bh = load_cvt(root1_d[CO:cin, :], [CO, CO], "root1bh")
            root2b = load_cvt(root2_d[:, :], [CO, CO], "root2b")

            dst_sl = persist.tile([WIN, ttot * 2], F16, tag="dst")
            nc.sync.dma_start(dst_sl[:], dstr_d[:, :])
            src_sl = persist.tile([WIN, ttot * 8], I16, tag="src16")
            nc.sync.dma_start(src_sl[:], srcs_d[:, :])
            # transposed window accumulator slab [64, nw*128] f32
            hacc = persist.tile([CO, nw * WIN], F32, tag="hacc")

            # skipT f32 -> skipTbf DRAM bf16 (chunks)
            for c0 in range(0, npad, ch):
                t_f = work.tile([CO, ch], F32, tag="rcf")
                nc.sync.dma_start(t_f[:], skipT[:, c0 : c0 + ch])
                t_b = work.tile([CO, ch], BF16, tag="rcb")
                nc.vector.tensor_copy(t_b[:], t_f[:])
                nc.sync.dma_start(skipTbf[:, c0 : c0 + ch], t_b[:])

            # betas slab [128, ttot*9] bf16
            betas = persist.tile([WIN, ttot * S], BF16, tag="betas")
            TCB = cfg.tc_betas
            for c0 in range(0, ttot, TCB):
                bt = min(TCB, ttot - c0)
                ac = work.tile([WIN, TCB * 2], F32, tag="attr")
                nc.sync.dma_start(
                    ac[:, : bt * 2], attr2_d[:, c0 * 2 : (c0 + bt) * 2]
                )
                acv = ac[:, : bt * 2].rearrange("p (t c) -> p t c", c=2)
                bsp = []
                for dim in range(2):
                    u = acv[:, :, dim]
                    u2 = work.tile([WIN, TCB], F32, tag=f"u2_{dim}")
                    nc.vector.tensor_tensor(u2[:, :bt], u, u, op=OP.mult)
                    b = work.tile([WIN, TCB * 3], F32, tag=f"bsp_{dim}")
                    bv = b[:, : bt * 3].rearrange("p (t k) -> p t k", k=3)
                    nc.vector.tensor_scalar(
                        out=bv[:, :, 2], in0=u2[:, :bt], scalar1=0.5, scalar2=None,
                        op0=OP.mult,
                    )
                    nc.vector.tensor_tensor(
                        bv[:, :, 0], bv[:, :, 2], u, op=OP.subtract
                    )
                    nc.vector.tensor_scalar(
                        out=bv[:, :, 0], in0=bv[:, :, 0], scalar1=0.5, scalar2=None,
                        op0=OP.add,
                    )
                    nc.vector.tensor_tensor(bv[:, :, 1], u, u2[:, :bt], op=OP.subtract)
                    nc.vector.tensor_scalar(
                        out=bv[:, :, 1], in0=bv[:, :, 1], scalar1=0.5, scalar2=None,
                        op0=OP.add,
                    )
                    bsp.append(bv)
                b0, b1 = bsp
                in0 = b0.rearrange("p t (x k) -> p t x k", x=1).to_broadcast(
                    [WIN, bt, 3, 3]
                )
                in1 = b1.rearrange("p t (k x) -> p t k x", x=1).to_broadcast(
                    [WIN, bt, 3, 3]
                )
                outv = betas[:, c0 * S : (c0 + bt) * S].rearrange(
                    "p (t a b) -> p t a b", a=3, b=3
                )
                nc.vector.tensor_tensor(outv, in0, in1, op=OP.mult)

            tc.strict_bb_all_engine_barrier()

            # ------------- layers -------------
            def table_unit(li, blk, cc):
                table = tables[li]
                if True:
                    if True:
                        n0 = blk * shp + cc * ch
                        if li == 0:
                            xc_f = work.tile([cin, ch], F32, tag="rcf")
                            nc.sync.dma_start(xc_f[:], xT[:, n0 : n0 + ch])
                            xc = work.tile([cin, ch], BF16, tag="rcb")
                            nc.vector.tensor_copy(xc[:], xc_f[:])
                            halves = [(xc, W1b, None)]
                        elif li == 1:
                            # stack [h; skip] into one [128, ch] tile -> one
                            # K=128 matmul per column split instead of two
                            # K=64 halves
                            hsc = work.tile([cin, ch], BF16, tag="rcb2")
                            nc.sync.dma_start(
                                hsc[0:CO, :],
                                hstackT[0][
                                    blk * CO : (blk + 1) * CO, cc * ch : (cc + 1) * ch
                                ],
                            )
                            nc.sync.dma_start(
                                hsc[CO:cin, :], skipTbf[:, n0 : n0 + ch]
                            )
                            halves = [(hsc, W1b, None)]
                        else:
                            hc = work.tile([CO, ch], BF16, tag="rcb2")
                            nc.sync.dma_start(
                                hc[:],
                                hstackT[1][
                                    blk * CO : (blk + 1) * CO, cc * ch : (cc + 1) * ch
                                ],
                            )
                            halves = [(hc, W2b, None)]
                        for ti in range(ntch):
                            ps = psum.tile([WIN, ROWP], F32, tag="pstab")
                            sl = slice(ti * WIN, (ti + 1) * WIN)
                            for c0, c1 in ((0, 512), (512, ROWW)):
                                for hi, (lh, wb, rows) in enumerate(halves):
                                    rhs = (
                                        wb[rows, c0:c1]
                                        if rows is not None
                                        else wb[:, c0:c1]
                                    )
                                    nc.tensor.matmul(
                                        ps[:, c0:c1],
                                        lhsT=lh[:, sl],
                                        rhs=rhs,
                                        start=(hi == 0),
                                        stop=(hi == len(halves) - 1),
                                    )
                            tb = work.tile([WIN, ROWW], BF16, tag="tbb")
                            nc.scalar.activation(tb[:], ps[:, :ROWW], AF.Copy)
                            nc.sync.dma_start(
                                table[n0 + ti * WIN : n0 + (ti + 1) * WIN, :ROWW],
                                tb[:],
                            )

            def table_units(li, blks):
                return [
                    (lambda blk=blk, cc=cc: table_unit(li, blk, cc))
                    for blk in blks
                    for cc in range(nchunks)
                ]

            def edge_group(li, g, units):
                table = tables[li]
                GB, BD = cfg.gb, cfg.bd
                if True:
                    t0g, t1g = int(voff[g][0]), int(voff[g][nv])
                    ngt = t1g - t0g
                    gbase = dbase = 0
                    g_tile = d_m = None
                    for v in range(nv):
                        ntv = int(tcv[g][v])
                        nwin = min(4, nw - 4 * v)  # 128-dst halves in window
                        psw = psumw.tile([CO, 512], F32, tag="psw")
                        for j in range(ntv):
                            t = int(voff[g][v]) + j
                            tt = t - t0g
                            if tt % GB == 0:
                                bt = min(GB, ngt - tt)
                                g_tile = gpool.tile([WIN, GB * ROWP], BF16, tag="g")
                                gbase = tt
                                nc.gpsimd.dma_gather(
                                    out_ap=g_tile[:, : bt * ROWP].rearrange(
                                        "p (t c) -> p t c", c=ROWP
                                    ),
                                    in_ap=table[g * grows : (g + 1) * grows, :],
                                    idxs_ap=src_sl[:, t0g * 8 + tt * 8 : t0g * 8 + (tt + bt) * 8],
                                    num_idxs=bt * WIN,
                                    num_idxs_reg=bt * WIN,
                                    elem_size=ROWP,
                                    single_packet=False,
                                    queue_num=(tt // GB) % 4,
                                )
                            if tt % BD == 0:
                                bt = min(BD, ngt - tt)
                                dbase = tt
                                go = tt - gbase
                                gsrc = (
                                    g_tile[:, go * ROWP : (go + bt) * ROWP]
                                    .rearrange("p (t c) -> p t c", c=ROWP)[:, :, :ROWW]
                                    .rearrange("p t (o s) -> p t o s", o=CO, s=S)
                                )
                                bv = (
                                    betas[:, t * S : (t + bt) * S]
                                    .rearrange("p (t s) -> p t s", s=S)
                                    .rearrange("p t (x s) -> p t x s", x=1)
                                    .to_broadcast([WIN, bt, CO, S])
                                )
                                q = dpool.tile([WIN, BD * ROWW], BF16, tag="q")
                                qv = q[:, : bt * ROWW].rearrange(
                                    "p (t o s) -> p t o s", o=CO, s=S
                                )
                                nc.vector.tensor_tensor(qv, gsrc, bv, op=OP.mult)
                                # bf16 add-tree over the 9 spline slots,
                                # in place in q's s-slots:
                                # q[0:4]+=q[4:8]; q[4:6]=q[0:2]+q[2:4];
                                # q[6]=q[4]+q[5]; m=q[6]+q[8]
                                nc.vector.tensor_tensor(
                                    qv[:, :, :, 0:4],
                                    qv[:, :, :, 0:4],
                                    qv[:, :, :, 4:8],
                                    op=OP.add,
                                )
                                nc.vector.tensor_tensor(
                                    qv[:, :, :, 4:6],
                                    qv[:, :, :, 0:2],
                                    qv[:, :, :, 2:4],
                                    op=OP.add,
                                )
                                nc.vector.tensor_tensor(
                                    qv[:, :, :, 6:7],
                                    qv[:, :, :, 4:5],
                                    qv[:, :, :, 5:6],
                                    op=OP.add,
                                )
                                d_m = dpool.tile([WIN, BD * CO], F16, tag="m")
                                nc.vector.tensor_tensor(
                                    d_m[:, : bt * CO].rearrange(
                                        "p (t o k) -> p t o k", o=CO, k=1
                                    ),
                                    qv[:, :, :, 6:7],
                                    qv[:, :, :, 8:9],
                                    op=OP.add,
                                )
                                # batched dst one-hot: iota [p,(256,2)] fp16
                                # vs 2-replicated dst (packed last dim -> 2x)
                                d_sel = dpool.tile([WIN, BD * 512], F16, tag="sel")
                                nc.vector.tensor_tensor(
                                    d_sel[:, : bt * 512].rearrange(
                                        "p (t g k) -> p t g k", g=256, k=2
                                    ),
                                    iota_t[:]
                                    .rearrange("p (x g k) -> p x g k", x=1, k=2)
                                    .to_broadcast([WIN, bt, 256, 2]),
                                    dst_sl[:, t * 2 : (t + bt) * 2]
                                    .rearrange("p (t x k) -> p t x k", x=1, k=2)
                                    .to_broadcast([WIN, bt, 256, 2]),
                                    op=OP.is_equal,
                                )
                            i = tt - dbase
                            # one transposed scatter matmul: psw[o, d] +=
                            # m^T @ d_sel  (d_m stationary, 512-wide rhs)
                            nc.tensor.matmul(
                                psw[:],
                                lhsT=d_m[:, i * CO : (i + 1) * CO],
                                rhs=d_sel[:, i * 512 : (i + 1) * 512],
                                start=(j == 0),
                                stop=(j == ntv - 1),
                            )
                        # flush window into hacc (one contiguous add)
                        nc.vector.tensor_tensor(
                            hacc[:, 4 * v * WIN : (4 * v + nwin) * WIN],
                            hacc[:, 4 * v * WIN : (4 * v + nwin) * WIN],
                            psw[:, : nwin * WIN],
                            op=OP.add,
                        )
                        # interleave next stage's table chunks between windows
                        if units:
                            units.pop(0)()
                for u in units:
                    u()

            def edge_root(li):
                rootb = root1b if li < 2 else root2b
                rw = ch // WIN
                nc.vector.memset(hacc[:], 0.0)
                # root pass: hacc[w] += h_own @ root
                rootc = None
                skownc = None
                rootc_cc = -1
                for w in range(nw):
                    cc = w // rw
                    if cc != rootc_cc:
                        rootc_cc = cc
                        if li == 0:
                            rc_f = work.tile([cin, ch], F32, tag="rcf")
                            nc.sync.dma_start(
                                rc_f[:], xTown[:, cc * ch : (cc + 1) * ch]
                            )
                            rootc = work.tile([cin, ch], BF16, tag="rcb")
                            nc.vector.tensor_copy(rootc[:], rc_f[:])
                        else:
                            rootc = work.tile([CO, ch], BF16, tag="rcb2")
                            nc.sync.dma_start(
                                rootc[:],
                                bounceT[li - 1][:, cc * ch : (cc + 1) * ch],
                            )
                            if li == 1:
                                sk_f = work.tile([CO, ch], F32, tag="rcf")
                                nc.sync.dma_start(
                                    sk_f[:], skipTown[:, cc * ch : (cc + 1) * ch]
                                )
                                skownc = work.tile([CO, ch], BF16, tag="rcb")
                                nc.vector.tensor_copy(skownc[:], sk_f[:])
                    wsl = slice((w % rw) * WIN, (w % rw + 1) * WIN)
                    # transposed root: psr[o, n] = root^T @ x-chunk
                    psr = psum1.tile([CO, WIN], F32, tag="ps2")
                    if li == 1:
                        nc.tensor.matmul(
                            psr[:], lhsT=rootb[0:CO, :], rhs=rootc[:, wsl],
                            start=True, stop=False,
                        )
                        nc.tensor.matmul(
                            psr[:], lhsT=root1bh[:, :], rhs=skownc[:, wsl],
                            start=False, stop=True,
                        )
                    else:
                        nc.tensor.matmul(
                            psr[:], lhsT=rootb[:, :], rhs=rootc[:, wsl],
                            start=True, stop=True,
                        )
                    nc.vector.tensor_tensor(
                        hacc[:, w * WIN : (w + 1) * WIN],
                        hacc[:, w * WIN : (w + 1) * WIN],
                        psr[:],
                        op=OP.add,
                    )

            def edge_final(li):
                if li < 2:
                    # hacc is already [64, shp]: relu + cast, DMA straight out
                    for c0 in range(0, shp, ch):
                        hb = work.tile([CO, ch], BF16, tag="hb")
                        nc.scalar.activation(hb[:], hacc[:, c0 : c0 + ch], AF.Relu)
                        nc.sync.dma_start(bounceT[li][:, c0 : c0 + ch], hb[:])
                else:
                    # output wants [shp, 64]: transpose each 128-dst window
                    for w in range(nw):
                        hb = work.tile([CO, WIN], BF16, tag="hb")
                        nc.scalar.activation(
                            hb[:], hacc[:, w * WIN : (w + 1) * WIN], AF.Relu
                        )
                        pst = psum1.tile([WIN, CO], BF16, tag="pst")
                        nc.tensor.transpose(
                            out=pst[:], in_=hb[:], identity=ident[0:CO, 0:CO]
                        )
                        ho = work.tile([WIN, CO], F32, tag="ho")
                        nc.scalar.activation(ho[:], pst[:], AF.Copy)
                        nc.sync.dma_start(
                            out_d[w * WIN : (w + 1) * WIN, :], ho[:]
                        )

            for li in range(3):
                # stage -1: first two table blocks + hacc init/root terms
                for u in table_units(li, [0, 1]):
                    u()
                edge_root(li)
                tc.strict_bb_all_engine_barrier()
                # stages 0..ngrp-1: edge group g + table blocks for group g+1
                for g in range(ngrp):
                    blks = [2 * g + 2, 2 * g + 3] if g < ngrp - 1 else []
                    edge_group(li, g, table_units(li, blks))
                    tc.strict_bb_all_engine_barrier()
                edge_final(li)
                tc.strict_bb_all_engine_barrier()
                if li < 2:
                    nc.gpsimd.collective_compute(
                        "AllGather",
                        OP.bypass,
                        replica_groups=[list(range(cfg.ncores))],
                        ins=[bounceT[li].ap().opt()],
                        outs=[hstackT[li].ap().opt()],
                    )
                    tc.strict_bb_all_engine_barrier()
    nc.finalize()
    return nc


# --------------------------------------------------------------------------
# entry point
# --------------------------------------------------------------------------
def run_full(inputs, trace=False, trace_kwargs=None):
    cfg = FULL
    in_maps, sched = host_prep(
        cfg,
        np.asarray(inputs["x"], np.float32),
        np.asarray(inputs["skip"], np.float32),
        inputs["edge_index"],
        np.asarray(inputs["edge_attr"], np.float32),
        inputs["W1"],
        inputs["root1"],
        inputs["W2"],
        inputs["root2"],
    )
    nc = build_program(cfg, sched)
    from concourse.bass_utils import run_bass_kernel_spmd

    res = run_bass_kernel_spmd(
        nc,
        in_maps,
        core_ids=list(range(cfg.ncores)),
        trace=trace,
        **(dict(trace_kwargs=trace_kwargs) if trace_kwargs else {}),
    )
    out = np.zeros((cfg.n, CO), np.float32)
    for m in range(cfg.ncores):
        shard = res.results[m]["out_shard"]
        out[m * cfg.nsh : (m + 1) * cfg.nsh] = shard[: cfg.nsh]
    return out, res


def kernel(**inputs):
    out, _ = run_full(inputs)
    return out



# revision 62
# speedup vs baseline: 1.1280x; 1.1280x over previous
"""Trainium2 Bass kernel for nn_MeshUpConv (3x chained SplineConv, deg-2 2D
B-spline, N=100k nodes, E=1.6M edges) on 8 NeuronCores.

Strategy (destination-bucketed graph parallel):
  - Host: bucket edges by destination-owner core; within a core sort by
    (source-block, local dst). Per (core, source-block) the edge count is
    padded to whole 128-edge tiles with counts equalized across cores so the
    SPMD program is shared.
  - Device, per layer:
      * node phase: every core computes the FULL node-level transform table
        xW [Npad, 640] (bf16 rows: 576 = [o-major, s-minor] + 64 zero pad)
        via PE matmuls into its own DRAM.
      * edge phase, per source-block g: dma_gather (int16 block-local src
        ids, 2048 tokens/instr) pulls xW rows; DVE multiplies by the 9
        spline basis products (broadcast AP) and group-reduces; per-tile
        selection-matrix matmuls (1 or 2 windows, straddle-aware)
        scatter-add 128-edge tiles into per-window PSUM, flushed into an
        SBUF accumulator slab shared across blocks. Root terms ride block
        0's PSUM. Final: ReLU (+transpose for the next layer's
        feature-major consumption).
      * AllGather (partition-concat) of transposed shard outputs between
        layers.
All sharding metadata (bucketing, sorting, padding, index staging) is host
side; all numerics (basis evaluation, matmuls, gathers, reductions) on
device.
"""
import sys

sys.path.insert(0, "/opt/trn_rl_repo")

from contextlib import ExitStack
from dataclasses import dataclass

import numpy as np

import concourse.bass as bass
import concourse.tile as tile
from concourse import mybir
from concourse.masks import make_identity

F32 = mybir.dt.float32
BF16 = mybir.dt.bfloat16
F16 = mybir.dt.float16
I16 = mybir.dt.int16
AF = mybir.ActivationFunctionType
OP = mybir.AluOpType

S = 9
CO = 64
ROWW = S * CO  # 576 payload row width
ROWP = 640  # padded table row (1280 B, %256)
WIN = 128


@dataclass
class Cfg:
    ncores: int = 8
    nsh: int = 12500  # real nodes per core
    nw: int = 98  # 128-dst windows per core
    cin: int = 128
    ch: int = 896  # node-chunk (divides shp, multiple of 128)
    ngrp: int = 4  # source block-pair groups
    gb: int = 12  # tiles per dma_gather (1536 tokens)
    bd: int = 4  # DVE batch (tiles, divides gb)
    tc_betas: int = 128  # betas chunk (tiles)

    @property
    def shp(self):
        return self.nw * WIN

    @property
    def npad(self):
        return self.ncores * self.shp

    @property
    def n(self):
        return self.ncores * self.nsh


FULL = Cfg()


# --------------------------------------------------------------------------
# host-side schedule / sharding
# --------------------------------------------------------------------------
def host_prep(cfg, x, skip, edge_index, edge_attr, W1, root1, W2, root2):
    ncores, nsh, nw, shp = cfg.ncores, cfg.nsh, cfg.nw, cfg.shp
    ngrp = cfg.ngrp
    bpg = ncores // ngrp  # blocks per group (2)
    grows = bpg * shp  # table rows per group (25088)
    nv = (nw + 3) // 4  # 512-dst windows (25)
    src = np.asarray(edge_index[0]).astype(np.int64)
    dst = np.asarray(edge_index[1]).astype(np.int64)
    attr = np.asarray(edge_attr, dtype=np.float32)
    owner = dst // nsh
    dloc = dst - owner * nsh
    sblk = src // nsh
    grp = sblk // bpg
    # group-local padded source row
    sloc = (sblk % bpg) * shp + (src - sblk * nsh)
    vwin = dloc // 512

    # counts per (core, group, v) -> equalized tile counts
    cnt = np.zeros((ncores, ngrp, nv), np.int64)
    for m in range(ncores):
        for g in range(ngrp):
            sel = (owner == m) & (grp == g)
            cnt[m, g] = np.bincount(vwin[sel], minlength=nv)
    tcv = np.maximum(1, -(-cnt.max(axis=0) // WIN)).astype(np.int64)  # [ngrp, nv]
    gtc = tcv.sum(axis=1)  # tiles per group
    goff = np.concatenate([[0], np.cumsum(gtc)]).astype(np.int64)
    ttot = int(goff[-1])
    ne = ttot * WIN
    # tile offset of (g, v)
    voff = np.zeros((ngrp, nv + 1), np.int64)
    for g in range(ngrp):
        voff[g] = np.concatenate([[goff[g]], goff[g] + np.cumsum(tcv[g])])

    zero_local = nsh  # block-local pad row (zero); lives in first block of grp
    srcs = np.full((ncores, ne), zero_local, np.int64)
    dst_rel = np.full((ncores, ne), -1, np.int64)
    attrs = np.zeros((ncores, ne, 2), np.float32)
    for m in range(ncores):
        own = owner == m
        for g in range(ngrp):
            ing = own & (grp == g)
            for v in range(nv):
                sel = np.where(ing & (vwin == v))[0]
                k = len(sel)
                base = int(voff[g, v]) * WIN
                srcs[m, base : base + k] = sloc[sel]
                dst_rel[m, base : base + k] = dloc[sel] - v * 512
                attrs[m, base : base + k] = attr[sel]
    # per-edge window-relative dst, fp16 (exact for 0..511), replicated 4x so
    # the on-device is_equal keeps a packed last dim (DVE 2x mode); pad
    # edges get -1 (matches nothing)
    dst4 = np.repeat(dst_rel.astype(np.float16)[:, :, None], 4, axis=2)

    def tilize(a):
        a = a.reshape(ttot, WIN, *a.shape[1:])
        return np.ascontiguousarray(np.swapaxes(a, 0, 1))

    def idx_wrap(a):
        w = a.reshape(-1, 16).T.astype(np.int16)
        return np.ascontiguousarray(np.tile(w, (8, 1)))

    srcs_w = np.stack([idx_wrap(srcs[m]) for m in range(ncores)])
    dstr_t = np.stack(
        [tilize(dst4[m]).reshape(WIN, ttot * 4) for m in range(ncores)]
    )
    attr_t = np.stack(
        [tilize(attrs[m]).reshape(WIN, ttot * 2) for m in range(ncores)]
    )

    cin = cfg.cin
    xpad = np.zeros((cfg.npad, cin), np.float32)
    spad = np.zeros((cfg.npad, CO), np.float32)
    for m in range(ncores):
        xpad[m * shp : m * shp + nsh] = x[m * nsh : (m + 1) * nsh]
        spad[m * shp : m * shp + nsh] = skip[m * nsh : (m + 1) * nsh]
    xT = np.ascontiguousarray(xpad.T)
    skipT = np.ascontiguousarray(spad.T)

    def wall(W, fdim):
        w = np.transpose(np.asarray(W, np.float32), (1, 2, 0)).reshape(fdim, ROWW)
        return np.ascontiguousarray(
            np.concatenate([w, np.zeros((fdim, ROWP - ROWW), np.float32)], axis=1)
        )

    W1all = wall(W1, cin)
    W2all = wall(W2, CO)
    iota512 = np.ascontiguousarray(
        np.tile(np.arange(512, dtype=np.float16), (WIN, 1))
    )

    shared = dict(
        xT=xT,
        skipT=skipT,
        W1all=W1all,
        W2all=W2all,
        root1=np.asarray(root1, np.float32),
        root2=np.asarray(root2, np.float32),
        iota=iota512,
    )
    in_maps = []
    for m in range(ncores):
        d = dict(shared)
        d["xTown"] = np.ascontiguousarray(xT[:, m * shp : (m + 1) * shp])
        d["skipTown"] = np.ascontiguousarray(skipT[:, m * shp : (m + 1) * shp])
        d["srcs"] = srcs_w[m]
        d["dstr"] = dstr_t[m]
        d["attr2"] = attr_t[m]
        in_maps.append(d)
    sched = dict(
        ttot=ttot,
        tcv=[[int(v) for v in row] for row in tcv],
        voff=[[int(v) for v in row] for row in voff],
    )
    return in_maps, sched


# --------------------------------------------------------------------------
# device program
# --------------------------------------------------------------------------
def build_program(cfg, sched):
    from concourse import bacc

    nc = bacc.Bacc(
        "TRN2", target_bir_lowering=False, debug=False, num_devices=cfg.ncores
    )
    cin, shp, npad, nw = cfg.cin, cfg.shp, cfg.npad, cfg.nw
    ch = cfg.ch
    ntch = ch // WIN
    assert shp % ch == 0
    nchunks = shp // ch
    ttot = sched["ttot"]
    tcv = sched["tcv"]
    voff = sched["voff"]
    ngrp = cfg.ngrp
    grows = (cfg.ncores // ngrp) * shp
    nv = (nw + 3) // 4

    xT = nc.declare_dram_parameter("xT", [cin, npad], F32, isOutput=False)
    xTown = nc.declare_dram_parameter("xTown", [cin, shp], F32, isOutput=False)
    skipT = nc.declare_dram_parameter("skipT", [CO, npad], F32, isOutput=False)
    skipTown = nc.declare_dram_parameter("skipTown", [CO, shp], F32, isOutput=False)
    W1all_d = nc.declare_dram_parameter("W1all", [cin, ROWP], F32, isOutput=False)
    W2all_d = nc.declare_dram_parameter("W2all", [CO, ROWP], F32, isOutput=False)
    root1_d = nc.declare_dram_parameter("root1", [cin, CO], F32, isOutput=False)
    root2_d = nc.declare_dram_parameter("root2", [CO, CO], F32, isOutput=False)
    iota_d = nc.declare_dram_parameter("iota", [WIN, 512], F16, isOutput=False)
    srcs_d = nc.declare_dram_parameter(
        "srcs", [WIN, (ttot * WIN) // 16], I16, isOutput=False
    )
    dstr_d = nc.declare_dram_parameter("dstr", [WIN, ttot * 4], F16, isOutput=False)
    attr2_d = nc.declare_dram_parameter(
        "attr2", [WIN, ttot * 2], F32, isOutput=False
    )
    out_d = nc.declare_dram_parameter("out_shard", [shp, CO], F32, isOutput=True)

    tables = [nc.dram_tensor(f"table{i}", [npad, ROWP], BF16) for i in range(3)]
    bounceT = [nc.dram_tensor(f"bounceT{i}", [CO, shp], BF16) for i in range(2)]
    hstackT = [
        nc.dram_tensor(
            f"hstackT{i}", [cfg.ncores * CO, shp], BF16, addr_space="Shared"
        )
        for i in range(2)
    ]
    skipTbf = nc.dram_tensor("skipTbf", [CO, npad], BF16)

    with tile.TileContext(nc) as tc:
        with ExitStack() as ctx:
            persist = ctx.enter_context(tc.tile_pool(name="persist", bufs=1))
            work = ctx.enter_context(tc.tile_pool(name="work", bufs=2))
            gpool = ctx.enter_context(tc.tile_pool(name="gpool", bufs=3))
            spool = ctx.enter_context(tc.tile_pool(name="spool", bufs=3))
            dpool = ctx.enter_context(tc.tile_pool(name="dpool", bufs=2))
            psum = ctx.enter_context(tc.tile_pool(name="psum", bufs=2, space="PSUM"))
            psum1 = ctx.enter_context(tc.tile_pool(name="psum1", bufs=1, space="PSUM"))
            psumw = ctx.enter_context(tc.tile_pool(name="psumw", bufs=2, space="PSUM"))

            # ------------- prep -------------
            iota_t = persist.tile([WIN, 512], F16, tag="iota")
            nc.sync.dma_start(iota_t[:], iota_d[:, :])
            ident = persist.tile([WIN, WIN], BF16, tag="ident")
            make_identity(nc, ident[:])

            def load_cvt(dram_ap, shape, tag):
                tmp = work.tile(shape, F32, tag="wcvt")
                nc.sync.dma_start(tmp[:], dram_ap)
                out = persist.tile(shape, BF16, tag=tag)
                nc.vector.tensor_copy(out[:], tmp[:])
                return out

            W1b = load_cvt(W1all_d[:, :], [cin, ROWP], "W1b")
            W2b = load_cvt(W2all_d[:, :], [CO, ROWP], "W2b")
            root1b = load_cvt(root1_d[:, :], [cin, CO], "root1b")
            root1bh = load_cvt(root1_d[CO:cin, :], [CO, CO], "root1bh")
            root2b = load_cvt(root2_d[:, :], [CO, CO], "root2b")

            dst_sl = persist.tile([WIN, ttot * 4], F16, tag="dst")
            nc.sync.dma_start(dst_sl[:], dstr_d[:, :])
            # transposed window accumulator slab [64, nw*128] f32
            hacc = persist.tile([CO, nw * WIN], F32, tag="hacc")

            # skipT f32 -> skipTbf DRAM bf16 (chunks)
            for c0 in range(0, npad, ch):
                t_f = work.tile([CO, ch], F32, tag="rcf")
                nc.sync.dma_start(t_f[:], skipT[:, c0 : c0 + ch])
                t_b = work.tile([CO, ch], BF16, tag="rcb")
                nc.vector.tensor_copy(t_b[:], t_f[:])
                nc.sync.dma_start(skipTbf[:, c0 : c0 + ch], t_b[:])

            # betas slab [128, ttot*9] bf16
            betas = persist.tile([WIN, ttot * S], BF16, tag="betas")
            TCB = cfg.tc_betas
            for c0 in range(0, ttot, TCB):
                bt = min(TCB, ttot - c0)
                ac = work.tile([WIN, TCB * 2], F32, tag="attr")
                nc.sync.dma_start(
                    ac[:, : bt * 2], attr2_d[:, c0 * 2 : (c0 + bt) * 2]
                )
                acv = ac[:, : bt * 2].rearrange("p (t c) -> p t c", c=2)
                bsp = []
                for dim in range(2):
                    u = acv[:, :, dim]
                    u2 = work.tile([WIN, TCB], F32, tag=f"u2_{dim}")
                    nc.vector.tensor_tensor(u2[:, :bt], u, u, op=OP.mult)
                    b = work.tile([WIN, TCB * 3], F32, tag=f"bsp_{dim}")
                    bv = b[:, : bt * 3].rearrange("p (t k) -> p t k", k=3)
                    nc.vector.tensor_scalar(
                        out=bv[:, :, 2], in0=u2[:, :bt], scalar1=0.5, scalar2=None,
                        op0=OP.mult,
                    )
                    nc.vector.tensor_tensor(
                        bv[:, :, 0], bv[:, :, 2], u, op=OP.subtract
                    )
                    nc.vector.tensor_scalar(
                        out=bv[:, :, 0], in0=bv[:, :, 0], scalar1=0.5, scalar2=None,
                        op0=OP.add,
                    )
                    nc.vector.tensor_tensor(bv[:, :, 1], u, u2[:, :bt], op=OP.subtract)
                    nc.vector.tensor_scalar(
                        out=bv[:, :, 1], in0=bv[:, :, 1], scalar1=0.5, scalar2=None,
                        op0=OP.add,
                    )
                    bsp.append(bv)
                b0, b1 = bsp
                in0 = b0.rearrange("p t (x k) -> p t x k", x=1).to_broadcast(
                    [WIN, bt, 3, 3]
                )
                in1 = b1.rearrange("p t (k x) -> p t k x", x=1).to_broadcast(
                    [WIN, bt, 3, 3]
                )
                outv = betas[:, c0 * S : (c0 + bt) * S].rearrange(
                    "p (t a b) -> p t a b", a=3, b=3
                )
                nc.vector.tensor_tensor(outv, in0, in1, op=OP.mult)

            tc.strict_bb_all_engine_barrier()

            # ------------- layers -------------
            def table_unit(li, blk, cc):
                table = tables[li]
                if True:
                    if True:
                        n0 = blk * shp + cc * ch
                        if li == 0:
                            xc_f = work.tile([cin, ch], F32, tag="rcf")
                            nc.sync.dma_start(xc_f[:], xT[:, n0 : n0 + ch])
                            xc = work.tile([cin, ch], BF16, tag="rcb")
                            nc.vector.tensor_copy(xc[:], xc_f[:])
                            halves = [(xc, W1b, None)]
                        elif li == 1:
                            # stack [h; skip] into one [128, ch] tile -> one
                            # K=128 matmul per column split instead of two
                            # K=64 halves
                            hsc = work.tile([cin, ch], BF16, tag="rcb2")
                            nc.sync.dma_start(
                                hsc[0:CO, :],
                                hstackT[0][
                                    blk * CO : (blk + 1) * CO, cc * ch : (cc + 1) * ch
                                ],
                            )
                            nc.sync.dma_start(
                                hsc[CO:cin, :], skipTbf[:, n0 : n0 + ch]
                            )
                            halves = [(hsc, W1b, None)]
                        else:
                            hc = work.tile([CO, ch], BF16, tag="rcb2")
                            nc.sync.dma_start(
                                hc[:],
                                hstackT[1][
                                    blk * CO : (blk + 1) * CO, cc * ch : (cc + 1) * ch
                                ],
                            )
                            halves = [(hc, W2b, None)]
                        for ti in range(ntch):
                            ps = psum.tile([WIN, ROWP], F32, tag="pstab")
                            sl = slice(ti * WIN, (ti + 1) * WIN)
                            for c0, c1 in ((0, 512), (512, ROWW)):
                                for hi, (lh, wb, rows) in enumerate(halves):
                                    rhs = (
                                        wb[rows, c0:c1]
                                        if rows is not None
                                        else wb[:, c0:c1]
                                    )
                                    nc.tensor.matmul(
                                        ps[:, c0:c1],
                                        lhsT=lh[:, sl],
                                        rhs=rhs,
                                        start=(hi == 0),
                                        stop=(hi == len(halves) - 1),
                                    )
                            tb = work.tile([WIN, ROWW], BF16, tag="tbb")
                            nc.scalar.activation(tb[:], ps[:, :ROWW], AF.Copy)
                            nc.sync.dma_start(
                                table[n0 + ti * WIN : n0 + (ti + 1) * WIN, :ROWW],
                                tb[:],
                            )

            def table_units(li, blks):
                return [
                    (lambda blk=blk, cc=cc: table_unit(li, blk, cc))
                    for blk in blks
                    for cc in range(nchunks)
                ]

            def edge_group(li, g, units):
                table = tables[li]
                GB, BD = cfg.gb, cfg.bd
                if True:
                    t0g, t1g = int(voff[g][0]), int(voff[g][nv])
                    ngt = t1g - t0g
                    gbase = dbase = 0
                    g_tile = d_m = None
                    for v in range(nv):
                        ntv = int(tcv[g][v])
                        nwin = min(4, nw - 4 * v)  # 128-dst halves in window
                        psw = psumw.tile([CO, 512], F32, tag="psw")
                        for j in range(ntv):
                            t = int(voff[g][v]) + j
                            tt = t - t0g
                            if tt % GB == 0:
                                bt = min(GB, ngt - tt)
                                src_t = spool.tile([WIN, GB * 8], I16, tag="src")
                                nc.sync.dma_start(
                                    src_t[:, : bt * 8],
                                    srcs_d[:, t0g * 8 + tt * 8 : t0g * 8 + (tt + bt) * 8],
                                )
                                g_tile = gpool.tile([WIN, GB * ROWP], BF16, tag="g")
                                gbase = tt
                                nc.gpsimd.dma_gather(
                                    out_ap=g_tile[:, : bt * ROWP].rearrange(
                                        "p (t c) -> p t c", c=ROWP
                                    ),
                                    in_ap=table[g * grows : (g + 1) * grows, :],
                                    idxs_ap=src_t[:, : bt * 8],
                                    num_idxs=bt * WIN,
                                    num_idxs_reg=bt * WIN,
                                    elem_size=ROWP,
                                    single_packet=False,
                                )
                            if tt % BD == 0:
                                bt = min(BD, ngt - tt)
                                dbase = tt
                                go = tt - gbase
                                gsrc = (
                                    g_tile[:, go * ROWP : (go + bt) * ROWP]
                                    .rearrange("p (t c) -> p t c", c=ROWP)[:, :, :ROWW]
                                    .rearrange("p t (o s) -> p t o s", o=CO, s=S)
                                )
                                bv = (
                                    betas[:, t * S : (t + bt) * S]
                                    .rearrange("p (t s) -> p t s", s=S)
                                    .rearrange("p t (x s) -> p t x s", x=1)
                                    .to_broadcast([WIN, bt, CO, S])
                                )
                                q = dpool.tile([WIN, BD * ROWW], BF16, tag="q")
                                qv = q[:, : bt * ROWW].rearrange(
                                    "p (t o s) -> p t o s", o=CO, s=S
                                )
                                nc.vector.tensor_tensor(qv, gsrc, bv, op=OP.mult)
                                # bf16 add-tree over the 9 spline slots
                                t1 = dpool.tile([WIN, BD * CO * 4], BF16, tag="t1")
                                t1v = t1[:, : bt * CO * 4].rearrange(
                                    "p (t o k) -> p t o k", o=CO, k=4
                                )
                                nc.vector.tensor_tensor(
                                    t1v, qv[:, :, :, 0:4], qv[:, :, :, 4:8], op=OP.add
                                )
                                t2 = dpool.tile([WIN, BD * CO * 2], BF16, tag="t2")
                                t2v = t2[:, : bt * CO * 2].rearrange(
                                    "p (t o k) -> p t o k", o=CO, k=2
                                )
                                nc.vector.tensor_tensor(
                                    t2v, t1v[:, :, :, 0:2], t1v[:, :, :, 2:4], op=OP.add
                                )
                                m0 = dpool.tile([WIN, BD * CO], BF16, tag="m0")
                                m0v = m0[:, : bt * CO].rearrange(
                                    "p (t o k) -> p t o k", o=CO, k=1
                                )
                                nc.vector.tensor_tensor(
                                    m0v, t2v[:, :, :, 0:1], t2v[:, :, :, 1:2], op=OP.add
                                )
                                d_m = dpool.tile([WIN, BD * CO], F16, tag="m")
                                nc.vector.tensor_tensor(
                                    d_m[:, : bt * CO].rearrange(
                                        "p (t o k) -> p t o k", o=CO, k=1
                                    ),
                                    m0v,
                                    qv[:, :, :, 8:9],
                                    op=OP.add,
                                )
                                # batched dst one-hot: iota [p,(128,4)] fp16
                                # vs 4-replicated dst (packed last dim -> 2x)
                                d_sel = dpool.tile([WIN, BD * 512], F16, tag="sel")
                                nc.vector.tensor_tensor(
                                    d_sel[:, : bt * 512].rearrange(
                                        "p (t g k) -> p t g k", g=128, k=4
                                    ),
                                    iota_t[:]
                                    .rearrange("p (x g k) -> p x g k", x=1, k=4)
                                    .to_broadcast([WIN, bt, 128, 4]),
                                    dst_sl[:, t * 4 : (t + bt) * 4]
                                    .rearrange("p (t x k) -> p t x k", x=1, k=4)
                                    .to_broadcast([WIN, bt, 128, 4]),
                                    op=OP.is_equal,
                                )
                            i = tt - dbase
                            # one transposed scatter matmul: psw[o, d] +=
                            # m^T @ d_sel  (d_m stationary, 512-wide rhs)
                            nc.tensor.matmul(
                                psw[:],
                                lhsT=d_m[:, i * CO : (i + 1) * CO],
                                rhs=d_sel[:, i * 512 : (i + 1) * 512],
                                start=(j == 0),
                                stop=(j == ntv - 1),
                            )
                        # flush window into hacc (one contiguous add)
                        nc.vector.tensor_tensor(
                            hacc[:, 4 * v * WIN : (4 * v + nwin) * WIN],
                            hacc[:, 4 * v * WIN : (4 * v + nwin) * WIN],
                            psw[:, : nwin * WIN],
                            op=OP.add,
                        )
                        # interleave next stage's table chunks between windows
                        if units:
                            units.pop(0)()
                for u in units:
                    u()

            def edge_root(li):
                rootb = root1b if li < 2 else root2b
                rw = ch // WIN
                nc.vector.memset(hacc[:], 0.0)
                # root pass: hacc[w] += h_own @ root
                rootc = None
                skownc = None
                rootc_cc = -1
                for w in range(nw):
                    cc = w // rw
                    if cc != rootc_cc:
                        rootc_cc = cc
                        if li == 0:
                            rc_f = work.tile([cin, ch], F32, tag="rcf")
                            nc.sync.dma_start(
                                rc_f[:], xTown[:, cc * ch : (cc + 1) * ch]
                            )
                            rootc = work.tile([cin, ch], BF16, tag="rcb")
                            nc.vector.tensor_copy(rootc[:], rc_f[:])
                        else:
                            rootc = work.tile([CO, ch], BF16, tag="rcb2")
                            nc.sync.dma_start(
                                rootc[:],
                                bounceT[li - 1][:, cc * ch : (cc + 1) * ch],
                            )
                            if li == 1:
                                sk_f = work.tile([CO, ch], F32, tag="rcf")
                                nc.sync.dma_start(
                                    sk_f[:], skipTown[:, cc * ch : (cc + 1) * ch]
                                )
                                skownc = work.tile([CO, ch], BF16, tag="rcb")
                                nc.vector.tensor_copy(skownc[:], sk_f[:])
                    wsl = slice((w % rw) * WIN, (w % rw + 1) * WIN)
                    # transposed root: psr[o, n] = root^T @ x-chunk
                    psr = psum1.tile([CO, WIN], F32, tag="ps2")
                    if li == 1:
                        nc.tensor.matmul(
                            psr[:], lhsT=rootb[0:CO, :], rhs=rootc[:, wsl],
                            start=True, stop=False,
                        )
                        nc.tensor.matmul(
                            psr[:], lhsT=root1bh[:, :], rhs=skownc[:, wsl],
                            start=False, stop=True,
                        )
                    else:
                        nc.tensor.matmul(
                            psr[:], lhsT=rootb[:, :], rhs=rootc[:, wsl],
                            start=True, stop=True,
                        )
                    nc.vector.tensor_tensor(
                        hacc[:, w * WIN : (w + 1) * WIN],
                        hacc[:, w * WIN : (w + 1) * WIN],
                        psr[:],
                        op=OP.add,
                    )

            def edge_final(li):
                if li < 2:
                    # hacc is already [64, shp]: relu + cast, DMA straight out
                    for c0 in range(0, shp, ch):
                        hb = work.tile([CO, ch], BF16, tag="hb")
                        nc.scalar.activation(hb[:], hacc[:, c0 : c0 + ch], AF.Relu)
                        nc.sync.dma_start(bounceT[li][:, c0 : c0 + ch], hb[:])
                else:
                    # output wants [shp, 64]: transpose each 128-dst window
                    for w in range(nw):
                        hb = work.tile([CO, WIN], BF16, tag="hb")
                        nc.scalar.activation(
                            hb[:], hacc[:, w * WIN : (w + 1) * WIN], AF.Relu
                        )
                        pst = psum1.tile([WIN, CO], BF16, tag="pst")
                        nc.tensor.transpose(
                            out=pst[:], in_=hb[:], identity=ident[0:CO, 0:CO]
                        )
                        ho = work.tile([WIN, CO], F32, tag="ho")
                        nc.scalar.activation(ho[:], pst[:], AF.Copy)
                        nc.sync.dma_start(
                            out_d[w * WIN : (w + 1) * WIN, :], ho[:]
                        )

            for li in range(3):
                # stage -1: first two table blocks + hacc init/root terms
                for u in table_units(li, [0, 1]):
                    u()
                edge_root(li)
                tc.strict_bb_all_engine_barrier()
                # stages 0..ngrp-1: edge group g + table blocks for group g+1
                for g in range(ngrp):
                    blks = [2 * g + 2, 2 * g + 3] if g < ngrp - 1 else []
                    edge_group(li, g, table_units(li, blks))
                    if g == ngrp - 1:
                        edge_final(li)
                    tc.strict_bb_all_engine_barrier()
                if li < 2:
                    nc.gpsimd.collective_compute(
                        "AllGather",
                        OP.bypass,
                        replica_groups=[list(range(cfg.ncores))],
                        ins=[bounceT[li].ap().opt()],
                        outs=[hstackT[li].ap().opt()],
                    )
                    tc.strict_bb_all_engine_barrier()
    nc.finalize()
    return nc


# --------------------------------------------------------------------------
# entry point
# --------------------------------------------------------------------------
def run_full(inputs, trace=False, trace_kwargs=None):
    cfg = FULL
    in_maps, sched = host_prep(
        cfg,
        np.asarray(inputs["x"], np.float32),
        np.asarray(inputs["skip"], np.float32),
        inputs["edge_index"],
        np.asarray(inputs["edge_attr"], np.float32),
        inputs["W1"],
        inputs["root1"],
        inputs["W2"],
        inputs["root2"],
    )
    nc = build_program(cfg, sched)
    from concourse.bass_utils import run_bass_kernel_spmd

    res = run_bass_kernel_spmd(
        nc,
        in_maps,
        core_ids=list(range(cfg.ncores)),
        trace=trace,
        **(dict(trace_kwargs=trace_kwargs) if trace_kwargs else {}),
    )
    out = np.zeros((cfg.n, CO), np.float32)
    for m in range(cfg.ncores):
        shard = res.results[m]["out_shard"]
        out[m * cfg.nsh : (m + 1) * cfg.nsh] = shard[: cfg.nsh]
    return out, res


def kernel(**inputs):
    out, _ = run_full(inputs)
    return out

